# revision 4
# baseline (speedup 1.0000x reference)
"""GCGRU cell (graph-conv GRU, diffusion-conv gates) on 8 TRN2 NeuronCores.

Math (per batch b, N=1024 nodes, D=2 in-feats, U=64 units, S=2 supports):
  x0   = [H_b | inputs_b]                          (N, 66)  (feature-permuted)
  for gate g in {r, u, c}:
    pre_g = x0g @ Wg_m0 + sum_s A_s @ (x0g @ Wg_{m=s+1}) + bias_g
  (reassociated: (A_s @ x0) @ W == A_s @ (x0 @ W), so the N x N supports
   multiply a tiny (N, 64) matrix instead of the other association order)
  r, u = sigmoid(pre_r), sigmoid(pre_u); c = tanh(pre_c with x0c=[r*H|inputs])
  h = u * H + (1 - u) * c

Implementation notes:
  - Data parallel over batch: 32 batches -> 4 per core, no collectives.
  - supports[b] is cast f32->bf16 *during* the HBM->SBUF DMA (SWDGE cast)
    in natural layout (i on partitions, (j,s) free).
  - The j-contraction needs j on partitions, so A is transposed 128x128
    tile-wise on the TensorEngine (transpose-mode matmul with a bf16
    identity).  Transpose-mode permits bf16 PSUM output, so eight
    transposed tiles share one [128,1024] bf16 PSUM bank and drain in a
    single all-bf16 DVE copy (2x DVE rate) -- drains cost ~1/3 of the
    f32 variant.
  - Gate pre-activations accumulate in f32 PSUM over 512-col (full-bank)
    moving slices of A^T; sigmoid/tanh run on the ScalarEngine straight
    out of PSUM.  x0 and h ride an all-bf16 cast+transpose path.
"""

import numpy as np

import concourse.bacc as bacc
import concourse.mybir as mybir
import concourse.tile as tile
from concourse.bass_utils import run_bass_kernel_spmd
from concourse.masks import make_identity

B, N, D, U, S = 32, 1024, 2, 64, 2
F = D + U                      # 66
NCORES = 8
BPC = B // NCORES              # 4 batches per core
P = 128                        # partitions
JB = N // P                    # 8 j-blocks per support
F32 = mybir.dt.float32
BF16 = mybir.dt.bfloat16

_COMPILED = {}


def _build():
    nc = bacc.Bacc("TRN2", target_bir_lowering=False, debug=False)

    t_inputs = nc.dram_tensor("inputs", [BPC, N, D], F32, kind="ExternalInput")
    t_supports = nc.dram_tensor("supports", [BPC, N, N, S], F32, kind="ExternalInput")
    t_hprev = nc.dram_tensor("h_prev", [BPC, N * U], F32, kind="ExternalInput")
    t_wk = {g: nc.dram_tensor(f"{g}_kernel", [F * 3, U], F32, kind="ExternalInput")
            for g in "ruc"}
    t_wb = {g: nc.dram_tensor(f"{g}_bias", [U], F32, kind="ExternalInput")
            for g in "ruc"}
    t_out = nc.dram_tensor("out", [BPC, N * U], F32, kind="ExternalOutput")

    QC = 2                 # i-tiles per load chunk
    NCH = N // (QC * P)    # 4 chunks per batch
    NQ = 512               # phase moving-slice width (one PSUM bank of f32)
    NIC = N // NQ          # 2 phase column-groups per batch

    with tile.TileContext(nc) as tc:
        with (
            tc.tile_pool(name="const", bufs=1) as constp,
            tc.tile_pool(name="wt", bufs=1) as wtp,
            tc.tile_pool(name="abf", bufs=5) as abfp,
            tc.tile_pool(name="at", bufs=2) as atp,
            tc.tile_pool(name="act", bufs=2) as actp,
            tc.tile_pool(name="psT", bufs=3, space="PSUM") as psT,
            tc.tile_pool(name="psB", bufs=2, space="PSUM") as psB,
            tc.tile_pool(name="psM", bufs=3, space="PSUM") as psM,
        ):
            # ---- constants ----
            id_bf = constp.tile([P, P], BF16, tag="id_bf")
            make_identity(nc, id_bf[:])

            # ---- gate weights, hop blocks, permuted to [H|inputs], bf16 ----
            # W rows are (f, m) pairs, m fastest: row f*3 + m.  One staging
            # DMA per gate (rows permuted to [H|inputs]), bf16 casts on DVE.
            wst = {}
            for g in "ruc":
                st = wtp.tile([F, 3 * U], F32, tag=f"wst_{g}", name=f"wst_{g}")
                src = t_wk[g].ap().rearrange("(f three) u -> f (three u)", three=3)
                nc.sync.dma_start(st[0:U, :], src[D:F, :])
                nc.sync.dma_start(st[U:F, :], src[0:D, :])
                wst[g] = st

            def w_block(g, m):
                return wst[g][:, m * U:(m + 1) * U]

            w0ru = wtp.tile([F, 2 * U], BF16, tag="w0ru")
            nc.vector.tensor_copy(w0ru[:, 0:U], w_block("r", 0))
            nc.vector.tensor_copy(w0ru[:, U:2 * U], w_block("u", 0))
            wru_s = []
            for s in range(S):
                w = wtp.tile([F, 2 * U], BF16, tag=f"wru{s}")
                nc.vector.tensor_copy(w[:, 0:U], w_block("r", s + 1))
                nc.vector.tensor_copy(w[:, U:2 * U], w_block("u", s + 1))
                wru_s.append(w)
            wc0 = wtp.tile([F, U], BF16, tag="wc0")
            nc.vector.tensor_copy(wc0[:], w_block("c", 0))
            wc_s = []
            for s in range(S):
                w = wtp.tile([F, U], BF16, tag=f"wcs{s}")
                nc.vector.tensor_copy(w[:], w_block("c", s + 1))
                wc_s.append(w)

            bias = {}
            for g in "ruc":
                bt = wtp.tile([U, 1], F32, tag=f"bias_{g}")
                nc.sync.dma_start(bt[:], t_wb[g].ap().rearrange("(u one) -> u one", one=1))
                bias[g] = bt

            sup4 = t_supports.ap().rearrange(
                "b (q p) j two -> b p q (j two)", p=P)

            for b in range(BPC):
                at = [atp.tile([P, JB * N], BF16, tag=f"at{s}", name=f"at{s}")
                      for s in range(S)]
                abts = []
                for ch in range(NCH):
                    ab = abfp.tile([P, QC * N * S], BF16, tag="abf", name="ab")
                    nc.gpsimd.dma_start(
                        ab[:], sup4[b, :, ch * QC:(ch + 1) * QC, :])
                    abts.append(ab)

                # ---- x0 = [H | inputs]; bf16 cast; bf16 transpose (x0Tb) ----
                x0n = actp.tile([P, JB * F], F32, tag="x0n")
                nc.sync.dma_start(
                    x0n[:].rearrange("p (jb f) -> p jb f", f=F)[:, :, 0:U],
                    t_hprev.ap()[b].rearrange("(jb p u) -> p jb u", p=P, u=U))
                nc.sync.dma_start(
                    x0n[:].rearrange("p (jb f) -> p jb f", f=F)[:, :, U:F],
                    t_inputs.ap()[b].rearrange("(jb p) d -> p jb d", p=P))
                x0nb = actp.tile([P, JB * F], BF16, tag="x0nb")
                nc.vector.tensor_copy(x0nb[:], x0n[:])
                px = psM.tile([F, JB * P], BF16, tag="psM", name="px")
                for jb in range(JB):
                    nc.tensor.matmul(
                        px[:, jb * P:(jb + 1) * P],
                        x0nb[:, jb * F:(jb + 1) * F],
                        id_bf[:], start=(jb == 0), stop=(jb == JB - 1),
                        is_transpose=True)
                x0Tb = actp.tile([F, N], BF16, tag="x0Tb")
                nc.scalar.copy(x0Tb[:], px[:])

                # ---- Z_ru_s = x0 @ [Wr_{s+1} | Wu_{s+1}]  (N, 128) bf16 ----
                zru = []
                for s in range(S):
                    z = actp.tile([P, JB * 2 * U], BF16, tag=f"zru{s}")
                    for hf in range(2):
                        pz = psM.tile([P, 4 * 2 * U], F32, tag="psM",
                                      name="pz")
                        for q in range(4):
                            nc.tensor.matmul(
                                pz[:, q * 2 * U:(q + 1) * 2 * U],
                                x0Tb[:, (4 * hf + q) * P:(4 * hf + q + 1) * P],
                                wru_s[s][:], start=(q == 0), stop=(q == 3))
                        nc.scalar.copy(
                            z[:, hf * 8 * U:(hf + 1) * 8 * U], pz[:])
                    zru.append(z)

                # ---- supports loads (cast f32->bf16 in-DMA) + transposes.
                #      Transpose-mode matmuls, 8 tiles per bf16 PSUM bank,
                #      one all-bf16 DVE drain per bank. ----
                def load_and_transpose(ch):
                    ab = abts[ch]
                    ab4 = ab[:].rearrange("p (q j two) -> p q j two", q=QC, two=2)
                    for s in range(S):
                        atv = at[s][:].rearrange("p (jb n) -> p jb n", n=N)
                        for q in range(QC):
                            pt = psT.tile([P, JB * P], BF16, tag="psT",
                                          name="pt")
                            for jb in range(JB):
                                nc.tensor.matmul(
                                    pt[:, jb * P:(jb + 1) * P],
                                    ab4[:, q, jb * P:(jb + 1) * P, s],
                                    id_bf[:],
                                    start=(jb == 0), stop=(jb == JB - 1),
                                    is_transpose=True)
                            c0 = (ch * QC + q) * P
                            nc.vector.tensor_copy(
                                atv[:, :, c0:c0 + P],
                                pt[:].rearrange("p (jb q) -> p jb q", q=P))

                rT = actp.tile([U, N], BF16, tag="rT")
                uT = actp.tile([U, N], F32, tag="uT")

                def phase1(ic):
                    # half-column group: needs chunks 2*ic, 2*ic+1 transposed
                    p1 = psB.tile([P, NQ], F32, tag="psB", name="p1")
                    k = 0
                    for s in range(S):
                        for jb in range(JB):
                            nc.tensor.matmul(
                                p1[:],
                                zru[s][:, jb * 2 * U:(jb + 1) * 2 * U],
                                at[s][:, jb * N + ic * NQ: jb * N + (ic + 1) * NQ],
                                start=(k == 0), stop=False)
                            k += 1
                    nc.tensor.matmul(
                        p1[:], w0ru[:], x0Tb[:, ic * NQ:(ic + 1) * NQ],
                        start=False, stop=True)
                    nc.scalar.activation(
                        rT[:, ic * NQ:(ic + 1) * NQ], p1[0:U, :],
                        mybir.ActivationFunctionType.Sigmoid, bias=bias["r"][:])
                    nc.scalar.activation(
                        uT[:, ic * NQ:(ic + 1) * NQ], p1[U:2 * U, :],
                        mybir.ActivationFunctionType.Sigmoid, bias=bias["u"][:])

                for ic in range(NIC):
                    for ch in range(2 * ic, 2 * ic + 2):
                        load_and_transpose(ch)
                    phase1(ic)

                # ---- x0c^T = [(r * H)^T | inputs^T] (bf16) ----
                x0cT = actp.tile([F, N], BF16, tag="x0cT")
                nc.vector.tensor_copy(x0cT[U:F, :], x0Tb[U:F, :])
                nc.vector.tensor_mul(x0cT[0:U, :], rT[:], x0Tb[0:U, :])

                # ---- Z_c_s = x0c @ Wc_{s+1}  (N, 64) bf16 ----
                zc = []
                for s in range(S):
                    z = actp.tile([P, JB * U], BF16, tag=f"zc{s}")
                    pz = psM.tile([P, 8 * U], F32, tag="psM", name="pzc")
                    for q in range(JB):
                        nc.tensor.matmul(
                            pz[:, q * U:(q + 1) * U],
                            x0cT[:, q * P:(q + 1) * P],
                            wc_s[s][:], start=(q == 0), stop=(q == JB - 1))
                    nc.scalar.copy(z[:], pz[:])
                    zc.append(z)

                # ---- phase 2: pre_c^T ----
                cT = actp.tile([U, N], F32, tag="cT")
                for ic in range(NIC):
                    p2 = psB.tile([U, NQ], F32, tag="psB", name="p2")
                    k = 0
                    for s in range(S):
                        for jb in range(JB):
                            nc.tensor.matmul(
                                p2[:],
                                zc[s][:, jb * U:(jb + 1) * U],
                                at[s][:, jb * N + ic * NQ: jb * N + (ic + 1) * NQ],
                                start=(k == 0), stop=False)
                            k += 1
                    nc.tensor.matmul(
                        p2[:], wc0[:], x0cT[:, ic * NQ:(ic + 1) * NQ],
                        start=False, stop=True)
                    nc.scalar.activation(
                        cT[:, ic * NQ:(ic + 1) * NQ], p2[:],
                        mybir.ActivationFunctionType.Tanh, bias=bias["c"][:])

                # ---- h^T = c^T + u^T * (H^T - c^T);  back to natural ----
                hT = actp.tile([U, N], F32, tag="hT")
                nc.vector.tensor_sub(hT[:], x0Tb[0:U, :], cT[:])
                nc.vector.tensor_mul(hT[:], hT[:], uT[:])
                nc.vector.tensor_add(hT[:], hT[:], cT[:])
                hTb = actp.tile([U, N], BF16, tag="hTb")
                nc.vector.tensor_copy(hTb[:], hT[:])
                ph = psM.tile([P, 2 * JB * U], BF16, tag="psM", name="ph")
                for jb in range(JB):
                    nc.tensor.matmul(
                        ph[:, jb * U:(jb + 1) * U],
                        hTb[:, jb * P:(jb + 1) * P],
                        id_bf[0:U, 0:U], start=(jb == 0), stop=(jb == JB - 1),
                        is_transpose=True)
                hnat = actp.tile([P, JB * U], F32, tag="hnat")
                nc.vector.tensor_copy(hnat[:], ph[:, 0:JB * U])
                nc.sync.dma_start(
                    t_out.ap()[b].rearrange("(jb p u) -> p jb u", p=P, u=U),
                    hnat[:].rearrange("p (jb u) -> p jb u", u=U))

    nc.finalize()
    return nc


def _make_in_maps(inputs):
    in_maps = []
    for c in range(NCORES):
        lo, hi = c * BPC, (c + 1) * BPC
        in_maps.append({
            "inputs": np.ascontiguousarray(inputs["inputs"][lo:hi], np.float32),
            "supports": np.ascontiguousarray(inputs["supports"][lo:hi], np.float32),
            "h_prev": np.ascontiguousarray(inputs["h_prev"][lo:hi], np.float32),
            "r_kernel": np.ascontiguousarray(inputs["r_kernel"], np.float32),
            "u_kernel": np.ascontiguousarray(inputs["u_kernel"], np.float32),
            "c_kernel": np.ascontiguousarray(inputs["c_kernel"], np.float32),
            "r_bias": np.ascontiguousarray(inputs["r_bias"], np.float32),
            "u_bias": np.ascontiguousarray(inputs["u_bias"], np.float32),
            "c_bias": np.ascontiguousarray(inputs["c_bias"], np.float32),
        })
    return in_maps


def kernel(**inputs):
    nc = _COMPILED.get("nc")
    if nc is None:
        nc = _COMPILED["nc"] = _build()

    in_maps = _make_in_maps(inputs)
    last_err = None
    for _ in range(3):
        try:
            res = run_bass_kernel_spmd(nc, in_maps, core_ids=list(range(NCORES)))
            out = np.concatenate(
                [np.asarray(res.results[c]["out"]) for c in range(NCORES)], axis=0)
            return out.astype(np.float32)
        except Exception as e:  # sporadic NRT_EXEC_UNIT_UNRECOVERABLE flakes
            last_err = e
    raise last_err


# revision 8
# speedup vs baseline: 1.1467x; 1.1467x over previous
"""GCGRU cell (graph-conv GRU, diffusion-conv gates) on 8 TRN2 NeuronCores.

Math (per batch b, N=1024 nodes, D=2 in-feats, U=64 units, S=2 supports):
  x0   = [H_b | inputs_b]                          (N, 66)  (feature-permuted)
  for gate g in {r, u, c}:
    pre_g = x0g @ Wg_m0 + sum_s A_s @ (x0g @ Wg_{m=s+1}) + bias_g
  (reassociated: (A_s @ x0) @ W == A_s @ (x0 @ W), so the N x N supports
   multiply a tiny (N, 64) matrix instead of the other association order)
  r, u = sigmoid(pre_r), sigmoid(pre_u); c = tanh(pre_c with x0c=[r*H|inputs])
  h = u * H + (1 - u) * c

Implementation notes:
  - Data parallel over batch: 32 batches -> 4 per core, no collectives.
  - supports[b] is cast f32->bf16 *during* the HBM->SBUF DMA (SWDGE cast)
    in natural layout (i on partitions, (j,s) free).
  - The j-contraction needs j on partitions, so A is transposed 128x128
    tile-wise on the TensorEngine (transpose-mode matmul with a bf16
    identity).  Transpose-mode permits bf16 PSUM output, so eight
    transposed tiles share one [128,1024] bf16 PSUM bank and drain in a
    single all-bf16 DVE copy (2x DVE rate).  The DVE queue carries ONLY
    these drains so PSUM recycling is never head-of-line blocked.
  - h_prev/inputs load contiguously (2KB runs, in-DMA bf16 cast) in a
    node%8-interleaved partition layout; the PE transposes it and the
    PSUM drain un-interleaves with a strided write AP.  All four
    batches' x0^T / Z_ru precompute in a prologue that fills the PE
    while the first supports chunks stream in.
  - Gate pre-activations accumulate in f32 PSUM over 512-col (full-bank)
    moving slices of A^T; sigmoid/tanh run on the ScalarEngine straight
    out of PSUM.  Gate elementwise runs on GpSimd, and the next batch's
    support loads are issued ahead of the previous batch's tail so the
    SWDGE queue never waits on compute.
"""

import numpy as np

import concourse.bacc as bacc
import concourse.mybir as mybir
import concourse.tile as tile
from concourse.bass_utils import run_bass_kernel_spmd
from concourse.masks import make_identity

B, N, D, U, S = 32, 1024, 2, 64, 2
F = D + U                      # 66
NCORES = 8
BPC = B // NCORES              # 4 batches per core
P = 128                        # partitions
JB = N // P                    # 8 j-blocks per support
K8 = N // P                    # 8 nodes per partition in contiguous layout
F32 = mybir.dt.float32
BF16 = mybir.dt.bfloat16

_COMPILED = {}


def _build():
    nc = bacc.Bacc("TRN2", target_bir_lowering=False, debug=False)

    t_inputs = nc.dram_tensor("inputs", [BPC, N, D], F32, kind="ExternalInput")
    t_supports = nc.dram_tensor("supports", [BPC, N, N, S], F32, kind="ExternalInput")
    t_hprev = nc.dram_tensor("h_prev", [BPC, N * U], F32, kind="ExternalInput")
    t_wk = {g: nc.dram_tensor(f"{g}_kernel", [F * 3, U], F32, kind="ExternalInput")
            for g in "ruc"}
    t_wb = {g: nc.dram_tensor(f"{g}_bias", [U], F32, kind="ExternalInput")
            for g in "ruc"}
    t_out = nc.dram_tensor("out", [BPC, N * U], F32, kind="ExternalOutput")

    QC = 2                 # i-tiles per load chunk
    NCH = N // (QC * P)    # 4 chunks per batch
    NQ = 512               # phase moving-slice width (one PSUM bank of f32)
    NIC = N // NQ          # 2 phase column-groups per batch

    with tile.TileContext(nc) as tc:
        with (
            tc.tile_pool(name="const", bufs=1) as constp,
            tc.tile_pool(name="wt", bufs=1) as wtp,
            tc.tile_pool(name="pre", bufs=BPC) as prep,
            tc.tile_pool(name="abf", bufs=5) as abfp,
            tc.tile_pool(name="at", bufs=2) as atp,
            tc.tile_pool(name="act", bufs=2) as actp,
            tc.tile_pool(name="psT", bufs=3, space="PSUM") as psT,
            tc.tile_pool(name="psB", bufs=2, space="PSUM") as psB,
            tc.tile_pool(name="psM", bufs=3, space="PSUM") as psM,
        ):
            # ---- constants ----
            id_bf = constp.tile([P, P], BF16, tag="id_bf")
            make_identity(nc, id_bf[:])

            # ---- gate weights, hop blocks, permuted to [H|inputs], bf16 ----
            # W rows are (f, m) pairs, m fastest: row f*3 + m.  One staging
            # DMA per gate (rows permuted to [H|inputs]), bf16 casts on DVE.
            wst = {}
            for g in "ruc":
                st = wtp.tile([F, 3 * U], F32, tag=f"wst_{g}", name=f"wst_{g}")
                src = t_wk[g].ap().rearrange("(f three) u -> f (three u)", three=3)
                nc.sync.dma_start(st[0:U, :], src[D:F, :])
                nc.sync.dma_start(st[U:F, :], src[0:D, :])
                wst[g] = st

            def w_block(g, m):
                return wst[g][:, m * U:(m + 1) * U]

            w0ru = wtp.tile([F, 2 * U], BF16, tag="w0ru")
            nc.vector.tensor_copy(w0ru[:, 0:U], w_block("r", 0))
            nc.vector.tensor_copy(w0ru[:, U:2 * U], w_block("u", 0))
            wru_s = []
            for s in range(S):
                w = wtp.tile([F, 2 * U], BF16, tag=f"wru{s}")
                nc.vector.tensor_copy(w[:, 0:U], w_block("r", s + 1))
                nc.vector.tensor_copy(w[:, U:2 * U], w_block("u", s + 1))
                wru_s.append(w)
            wc0 = wtp.tile([F, U], BF16, tag="wc0")
            nc.vector.tensor_copy(wc0[:], w_block("c", 0))
            wc_s = []
            for s in range(S):
                w = wtp.tile([F, U], BF16, tag=f"wcs{s}")
                nc.vector.tensor_copy(w[:], w_block("c", s + 1))
                wc_s.append(w)

            bias = {}
            for g in "ruc":
                bt = wtp.tile([U, 1], F32, tag=f"bias_{g}")
                nc.sync.dma_start(bt[:], t_wb[g].ap().rearrange("(u one) -> u one", one=1))
                bias[g] = bt

            sup4 = t_supports.ap().rearrange(
                "b (q p) j two -> b p q (j two)", p=P)

            # ---- prologue: x0^T and Z_ru for ALL batches ----
            # h_prev/inputs load contiguously (node n = 8p+k on partition p,
            # slot k) with in-DMA bf16 cast; 16 transpose-mode matmuls per
            # batch land [H^T | x^T] interleaved in one PSUM bank; the
            # scalar drain un-interleaves via a strided write AP.
            hcb, xcb = [], []
            for b in range(BPC):
                hc = prep.tile([P, K8 * U], BF16, tag="hcb", name="hcb")
                nc.gpsimd.dma_start(
                    hc[:], t_hprev.ap()[b].rearrange("(p m) -> p m", p=P))
                xc = prep.tile([P, K8 * D], BF16, tag="xcb", name="xcb")
                nc.gpsimd.dma_start(
                    xc[:], t_inputs.ap()[b].rearrange("(p k) d -> p (k d)", p=P))
                hcb.append(hc)
                xcb.append(xc)

            x0Tb_l, zru_l = [], []
            for b in range(BPC):
                px = psM.tile([F, JB * P], BF16, tag="psM", name="px")
                for k in range(K8):
                    nc.tensor.matmul(
                        px[0:U, k * P:(k + 1) * P],
                        hcb[b][:, k * U:(k + 1) * U],
                        id_bf[:], start=(k == 0), stop=False,
                        is_transpose=True)
                for k in range(K8):
                    nc.tensor.matmul(
                        px[U:F, k * P:(k + 1) * P],
                        xcb[b][:, k * D:(k + 1) * D],
                        id_bf[:], start=False, stop=(k == K8 - 1),
                        is_transpose=True)
                x0Tb = prep.tile([F, N], BF16, tag="x0Tb", name="x0Tb")
                nc.scalar.copy(
                    x0Tb[:].rearrange("f (p k) -> f k p", k=K8),
                    px[:].rearrange("f (k p) -> f k p", p=P))
                x0Tb_l.append(x0Tb)

                zru = []
                for s in range(S):
                    z = prep.tile([P, JB * 2 * U], BF16, tag=f"zru{s}",
                                  name=f"zru{s}")
                    for hf in range(2):
                        pz = psM.tile([P, 4 * 2 * U], F32, tag="psM",
                                      name="pz")
                        for q in range(4):
                            nc.tensor.matmul(
                                pz[:, q * 2 * U:(q + 1) * 2 * U],
                                x0Tb[:, (4 * hf + q) * P:(4 * hf + q + 1) * P],
                                wru_s[s][:], start=(q == 0), stop=(q == 3))
                        nc.scalar.copy(
                            z[:, hf * 8 * U:(hf + 1) * 8 * U], pz[:])
                    zru.append(z)
                zru_l.append(zru)

            # ---- per-batch state for the staged main loop ----
            state = {}

            def issue_loads(b):
                at = [atp.tile([P, JB * N], BF16, tag=f"at{s}", name=f"at{s}")
                      for s in range(S)]
                abts = []
                for ch in range(NCH):
                    ab = abfp.tile([P, QC * N * S], BF16, tag="abf", name="ab")
                    nc.gpsimd.dma_start(
                        ab[:], sup4[b, :, ch * QC:(ch + 1) * QC, :])
                    abts.append(ab)
                state[b] = {"at": at, "abts": abts}

            def transpose_and_phase1(b):
                st = state[b]
                at, abts = st["at"], st["abts"]
                x0Tb, zru = x0Tb_l[b], zru_l[b]

                rT = actp.tile([U, N], BF16, tag="rT", name="rT")
                uT = actp.tile([U, N], F32, tag="uT", name="uT")

                def load_and_transpose(ch):
                    ab4 = abts[ch][:].rearrange(
                        "p (q j two) -> p q j two", q=QC, two=2)
                    for s in range(S):
                        atv = at[s][:].rearrange("p (jb n) -> p jb n", n=N)
                        for q in range(QC):
                            pt = psT.tile([P, JB * P], BF16, tag="psT",
                                          name="pt")
                            for jb in range(JB):
                                nc.tensor.matmul(
                                    pt[:, jb * P:(jb + 1) * P],
                                    ab4[:, q, jb * P:(jb + 1) * P, s],
                                    id_bf[:],
                                    start=(jb == 0), stop=(jb == JB - 1),
                                    is_transpose=True)
                            c0 = (ch * QC + q) * P
                            nc.vector.tensor_copy(
                                atv[:, :, c0:c0 + P],
                                pt[:].rearrange("p (jb q) -> p jb q", q=P))

                def phase1(ic):
                    p1 = psB.tile([P, NQ], F32, tag="psB", name="p1")
                    k = 0
                    for s in range(S):
                        for jb in range(JB):
                            nc.tensor.matmul(
                                p1[:],
                                zru[s][:, jb * 2 * U:(jb + 1) * 2 * U],
                                at[s][:, jb * N + ic * NQ: jb * N + (ic + 1) * NQ],
                                start=(k == 0), stop=False)
                            k += 1
                    nc.tensor.matmul(
                        p1[:], w0ru[:], x0Tb[:, ic * NQ:(ic + 1) * NQ],
                        start=False, stop=True)
                    nc.scalar.activation(
                        rT[:, ic * NQ:(ic + 1) * NQ], p1[0:U, :],
                        mybir.ActivationFunctionType.Sigmoid, bias=bias["r"][:])
                    nc.scalar.activation(
                        uT[:, ic * NQ:(ic + 1) * NQ], p1[U:2 * U, :],
                        mybir.ActivationFunctionType.Sigmoid, bias=bias["u"][:])

                for ic in range(NIC):
                    for ch in range(2 * ic, 2 * ic + 2):
                        load_and_transpose(ch)
                    phase1(ic)
                st["rT"], st["uT"] = rT, uT

            def tail(b):
                st = state[b]
                at = st["at"]
                rT, uT = st["rT"], st["uT"]
                x0Tb = x0Tb_l[b]

                # precompute g1 = u*H^T and w = 1-u off the critical path
                g1 = actp.tile([U, N], F32, tag="g1", name="g1")
                nc.gpsimd.tensor_mul(g1[:], uT[:], x0Tb[0:U, :])
                wT = actp.tile([U, N], F32, tag="wT", name="wT")
                nc.gpsimd.tensor_scalar(wT[:], uT[:], -1.0, 1.0,
                                        mybir.AluOpType.mult,
                                        mybir.AluOpType.add)

                # x0c^T = [(r * H)^T | inputs^T] (bf16)
                x0cT = actp.tile([F, N], BF16, tag="x0cT", name="x0cT")
                nc.gpsimd.tensor_copy(x0cT[U:F, :], x0Tb[U:F, :])
                nc.gpsimd.tensor_mul(x0cT[0:U, :], rT[:], x0Tb[0:U, :])

                # Z_c_s = x0c @ Wc_{s+1}  (N, 64) bf16
                zc = []
                for s in range(S):
                    z = actp.tile([P, JB * U], BF16, tag=f"zc{s}",
                                  name=f"zc{s}")
                    pz = psM.tile([P, 8 * U], F32, tag="psM", name="pzc")
                    for q in range(JB):
                        nc.tensor.matmul(
                            pz[:, q * U:(q + 1) * U],
                            x0cT[:, q * P:(q + 1) * P],
                            wc_s[s][:], start=(q == 0), stop=(q == JB - 1))
                    nc.scalar.copy(z[:], pz[:])
                    zc.append(z)

                # phase 2 + h, pipelined per column half
                cT = actp.tile([U, N], F32, tag="cT", name="cT")
                hTb = actp.tile([U, N], BF16, tag="hTb", name="hTb")
                ph = psM.tile([P, 2 * JB * U], BF16, tag="psM", name="ph")
                for ic in range(NIC):
                    p2 = psB.tile([U, NQ], F32, tag="psB", name="p2")
                    k = 0
                    for s in range(S):
                        for jb in range(JB):
                            nc.tensor.matmul(
                                p2[:],
                                zc[s][:, jb * U:(jb + 1) * U],
                                at[s][:, jb * N + ic * NQ: jb * N + (ic + 1) * NQ],
                                start=(k == 0), stop=False)
                            k += 1
                    nc.tensor.matmul(
                        p2[:], wc0[:], x0cT[:, ic * NQ:(ic + 1) * NQ],
                        start=False, stop=True)
                    sl = slice(ic * NQ, (ic + 1) * NQ)
                    nc.scalar.activation(
                        cT[:, sl], p2[:],
                        mybir.ActivationFunctionType.Tanh, bias=bias["c"][:])
                    # h^T = c^T * (1-u^T) + u^T * H^T
                    nc.gpsimd.tensor_mul(cT[:, sl], cT[:, sl], wT[:, sl])
                    nc.gpsimd.tensor_add(hTb[:, sl], cT[:, sl], g1[:, sl])
                    for jb in range(4 * ic, 4 * ic + 4):
                        nc.tensor.matmul(
                            ph[:, jb * U:(jb + 1) * U],
                            hTb[:, jb * P:(jb + 1) * P],
                            id_bf[0:U, 0:U],
                            start=(jb == 0), stop=(jb == JB - 1),
                            is_transpose=True)
                hnat = actp.tile([P, JB * U], F32, tag="hnat", name="hnat")
                nc.scalar.copy(hnat[:], ph[:, 0:JB * U])
                nc.sync.dma_start(
                    t_out.ap()[b].rearrange("(jb p u) -> p jb u", p=P, u=U),
                    hnat[:].rearrange("p (jb u) -> p jb u", u=U))

            # ---- staged main loop: loads for b+1 are issued before the
            #      tail of b so SWDGE descgen never waits on compute ----
            issue_loads(0)
            transpose_and_phase1(0)
            for b in range(1, BPC):
                issue_loads(b)
                tail(b - 1)
                transpose_and_phase1(b)
            tail(BPC - 1)

    nc.finalize()
    return nc


def _make_in_maps(inputs):
    in_maps = []
    for c in range(NCORES):
        lo, hi = c * BPC, (c + 1) * BPC
        in_maps.append({
            "inputs": np.ascontiguousarray(inputs["inputs"][lo:hi], np.float32),
            "supports": np.ascontiguousarray(inputs["supports"][lo:hi], np.float32),
            "h_prev": np.ascontiguousarray(inputs["h_prev"][lo:hi], np.float32),
            "r_kernel": np.ascontiguousarray(inputs["r_kernel"], np.float32),
            "u_kernel": np.ascontiguousarray(inputs["u_kernel"], np.float32),
            "c_kernel": np.ascontiguousarray(inputs["c_kernel"], np.float32),
            "r_bias": np.ascontiguousarray(inputs["r_bias"], np.float32),
            "u_bias": np.ascontiguousarray(inputs["u_bias"], np.float32),
            "c_bias": np.ascontiguousarray(inputs["c_bias"], np.float32),
        })
    return in_maps


def kernel(**inputs):
    nc = _COMPILED.get("nc")
    if nc is None:
        nc = _COMPILED["nc"] = _build()

    in_maps = _make_in_maps(inputs)
    last_err = None
    for _ in range(3):
        try:
            res = run_bass_kernel_spmd(nc, in_maps, core_ids=list(range(NCORES)))
            out = np.concatenate(
                [np.asarray(res.results[c]["out"]) for c in range(NCORES)], axis=0)
            return out.astype(np.float32)
        except Exception as e:  # sporadic NRT_EXEC_UNIT_UNRECOVERABLE flakes
            last_err = e
    raise last_err
